# revision 39
# baseline (speedup 1.0000x reference)
"""Causal self-attention (T=4096, D=1024, H=16) on 8 TRN2 NeuronCores.

Sharding: tensor-parallel over heads. Core i owns heads (2i, 2i+1):
  - computes its 384-row slice of the QKV projection (bf16 matmuls),
  - causal attention for its 2 heads in transposed orientation
    (scores sT[tk, tq] so the AV contraction needs no transposes),
  - causal mask: scores/exp/AV column-restricted on diagonal tiles; the
    128-wide triangular chunk is zeroed post-exp with a DVE multiply
    (keeps the PE free of mask matmuls and the ACT stream minimal),
  - softmax denominators via a ones-column appended to V,
  - its 128-dim slice of the c_proj contraction -> partial output
    [1024, 4096] in bf16.
Host sums the 8 partial outputs (the "all-reduce"), transposes, adds b_proj.
"""

import math

import ml_dtypes
import numpy as np

import concourse.bass as bass
import concourse.mybir as mybir
import concourse.tile as tile
from concourse import bacc
from concourse.bass import ts
from concourse.bass_utils import run_bass_kernel_spmd
from concourse.masks import make_identity

F32 = mybir.dt.float32
BF16 = mybir.dt.bfloat16
Exp = mybir.ActivationFunctionType.Exp

T = 4096
DM = 1024
NCORES = 8
NW = 8          # tq windows of 512
TQW = 512
NKT = 32        # tk tiles of 128
CT = 8          # c (d_model) tiles of 128


def seg2(tile_ap, lo, width):
    """Two-segment view [128, 2, width] of a [128, 1024] tile: columns
    [lo, lo+width) and [512+lo, 512+lo+width) (the two head halves)."""
    return bass.AP(
        tensor=tile_ap.tensor,
        offset=tile_ap.offset + lo,
        ap=[tile_ap.ap[0], [512, 2], [1, width]],
    )


def build_nc():
    nc = bacc.Bacc(None, target_bir_lowering=False)

    xT = nc.dram_tensor("xT", [DM, T], BF16, kind="ExternalInput")
    wqkvT = nc.dram_tensor("wqkvT", [DM, 384], BF16, kind="ExternalInput")
    bqkv = nc.dram_tensor("bqkv", [128, 3], F32, kind="ExternalInput")
    wpT = nc.dram_tensor("wpT", [128, DM], BF16, kind="ExternalInput")
    trimask = nc.dram_tensor("trimask", [128, 256], BF16, kind="ExternalInput")
    out = nc.dram_tensor("out", [DM, T], BF16, kind="ExternalOutput")

    xT_r = xT.ap().rearrange("(ct p) t -> p ct t", p=128)
    wq_r = wqkvT.ap().rearrange("(ct p) j -> p ct j", p=128)
    out_r = out.ap().rearrange("(ot p) t -> p ot t", p=128)

    with tile.TileContext(nc) as tc:
        with (
            tc.tile_pool(name="const", bufs=1) as const,
            tc.tile_pool(name="xw", bufs=4) as xw_pool,
            tc.tile_pool(name="vtmp", bufs=2) as vtmp_pool,
            tc.tile_pool(name="pt", bufs=8) as pt_pool,
            tc.tile_pool(name="r2", bufs=2) as r2_pool,
            tc.tile_pool(name="rb", bufs=2) as rb_pool,
            tc.tile_pool(name="yn", bufs=4) as yn_pool,
            tc.tile_pool(name="ynhi", bufs=2) as ynhi_pool,
            tc.tile_pool(name="ob", bufs=3) as ob_pool,
            tc.tile_pool(name="ps_sc", bufs=2, space="PSUM") as ps_sc,
            tc.tile_pool(name="ps_av", bufs=2, space="PSUM") as ps_av,
            tc.tile_pool(name="ps_qk", bufs=1, space="PSUM") as ps_qk,
            tc.tile_pool(name="ps_ev", bufs=1, space="PSUM") as ps_ev,
            tc.tile_pool(name="rdram", bufs=2, space="DRAM") as dram_pool,
        ):
            # ---- constants (window-0 DMAs issued first: critical path) ----
            wq_sb = const.tile([128, CT, 384], BF16)
            nc.sync.dma_start(out=wq_sb[:, :, ts(0, 128)], in_=wq_r[:, :, ts(0, 128)])
            xw0 = xw_pool.tile([128, CT, TQW], BF16, tag="xw")
            for c2 in range(4):
                nc.sync.dma_start(
                    out=xw0[:, ts(c2, 2), :], in_=xT_r[:, ts(c2, 2), ts(0, TQW)]
                )
            xw_tiles0 = xw0
            bq_sb = const.tile([128, 3], F32)
            nc.sync.dma_start(out=bq_sb[:], in_=bqkv[:])
            nc.sync.dma_start(out=wq_sb[:, :, ts(1, 128)], in_=wq_r[:, :, ts(1, 128)])
            nc.sync.dma_start(out=wq_sb[:, :, ts(2, 128)], in_=wq_r[:, :, ts(2, 128)])
            wp_sb = const.tile([128, DM], BF16)
            nc.sync.dma_start(out=wp_sb[:], in_=wpT[:])
            tm_sb = const.tile([128, 2, 128], BF16)
            nc.sync.dma_start(out=tm_sb[:], in_=trimask.ap())
            ident = const.tile([128, 128], BF16)
            make_identity(nc, ident[:])
            ones65 = const.tile([65, 64], F32)
            nc.vector.memset(ones65[:], 1.0)
            qT = const.tile([128, T], BF16)
            kT = const.tile([128, T], BF16)
            v_aug = const.tile([128, NKT, 130], BF16)
            nc.vector.memset(v_aug[:], 1.0)

            # ---- filler units (QKV projection + output projection) ----
            def f_xw_dma(w):
                def go():
                    if w in xw_tiles:
                        return
                    xw = xw_pool.tile([128, CT, TQW], BF16, tag="xw")
                    nc.sync.dma_start(out=xw[:], in_=xT_r[:, :, ts(w, TQW)])
                    xw_tiles[w] = xw
                return go

            def f_qkv_rb(w, rb, part=None, pool=None, tag="qkv"):
                """One QKV output chain; part=0..3 emits 2 of the 8
                contraction matmuls. Quarter-granularity matters: a filler
                unit bigger than ~2 matmuls overshoots the per-key-tile ACT
                slack (1147-641 = 506ns) and stalls the exp stream."""
                def go():
                    xw = xw_tiles[w]
                    if part in (None, 0):
                        ps = (pool or ps_qk).tile([128, TQW], F32, tag=tag)
                        qkv_ps[(w, rb)] = ps
                    ps = qkv_ps[(w, rb)]
                    cts = range(CT) if part is None else range(2 * part, 2 * part + 2)
                    for ct in cts:
                        nc.tensor.matmul(
                            ps[:],
                            wq_sb[:, ct, ts(rb, 128)],
                            xw[:, ct, :],
                            start=(ct == 0),
                            stop=(ct == CT - 1),
                        )
                    if part not in (None, 3):
                        return
                    del qkv_ps[(w, rb)]
                    if rb == 0:
                        nc.vector.tensor_scalar_add(
                            out=qT[:, ts(w, TQW)], in0=ps[:], scalar1=bq_sb[:, 0:1]
                        )
                    elif rb == 1:
                        nc.vector.tensor_scalar_add(
                            out=kT[:, ts(w, TQW)], in0=ps[:], scalar1=bq_sb[:, 1:2]
                        )
                    else:
                        vt = vtmp_pool.tile([128, TQW], BF16, tag="vt")
                        nc.vector.tensor_scalar_add(
                            out=vt[:], in0=ps[:], scalar1=bq_sb[:, 2:3]
                        )
                        vt_tiles[w] = vt
                return go

            def f_vtrans(w, k):
                def go():
                    vt = vt_tiles[w]
                    i = 4 * w + k
                    pst = ps_ev.tile([128, 128], BF16, tag="ev")
                    nc.tensor.transpose(pst[:], vt[:, ts(k, 128)], ident[:])
                    nc.vector.tensor_copy(out=v_aug[:, i, 0:64], in_=pst[:, 0:64])
                    nc.vector.tensor_copy(out=v_aug[:, i, 65:129], in_=pst[:, 64:128])
                return go

            def f_proj(j, ot, pool=None, tag="ev"):
                def go():
                    yn = yn_tiles[j]
                    pp = (pool or ps_ev).tile([128, TQW], F32, tag=tag)
                    nc.tensor.matmul(
                        pp[:], wp_sb[:, ts(ot, 128)], yn[:], start=True, stop=True
                    )
                    if ot == 0:
                        ob = ob_pool.tile([128, 8, TQW], BF16, tag="ob")
                        ob_tiles[j] = ob
                    ob = ob_tiles[j]
                    nc.vector.tensor_copy(out=ob[:, ot, :], in_=pp[:])
                    # two half-window output DMAs: the first half ships while
                    # the second half's matmuls still run, so the final
                    # window's out-write barely extends past the last cast
                    if ot == 3:
                        nc.sync.dma_start(
                            out=out_r[:, 0:4, ts(j, TQW)], in_=ob[:, 0:4, :]
                        )
                    elif ot == 7:
                        nc.sync.dma_start(
                            out=out_r[:, 4:8, ts(j, TQW)], in_=ob[:, 4:8, :]
                        )
                return go

            xw_tiles = {}
            vt_tiles = {}
            yn_tiles = {}
            ob_tiles = {}
            r2_tiles = {}
            qkv_ps = {}

            def emit_attn(j, fillers, prefillers=()):
                """Window j: scores+exp+mask+AV loop with fillers interleaved.

                Diagonal key tiles (i >= 4j, k = i-4j) are column-restricted
                to [128k, 512): columns left of that are fully masked. The
                remaining 128-wide triangular chunk is zeroed post-exp by a
                DVE multiply with the [128, 128] triangle.

                No fillers before iteration 2 — fillers may depend on the
                previous window's tail chain or fresh DMAs, and a blocked
                filler blocks everything behind it in PE program order.
                """
                for pf in prefillers:
                    pf()
                ntk = 4 * (j + 1)
                yh0 = ps_av.tile([65, TQW], F32, tag="av")
                yh1 = ps_av.tile([65, TQW], F32, tag="av")
                pts = {}
                los = {}
                nf = len(fillers)
                fi = 0
                span = max(1, ntk - 2)

                def emit_av(i):
                    lo = los[i]
                    nc.tensor.matmul(
                        yh0[:, lo:TQW],
                        v_aug[:, i, 0:65],
                        pts[i][:, lo:512],
                        start=(i == 0),
                        stop=(i == ntk - 1),
                        skip_group_check=True,
                    )
                    nc.tensor.matmul(
                        yh1[:, lo:TQW],
                        v_aug[:, i, 65:130],
                        pts[i][:, 512 + lo:1024],
                        start=(i == 0),
                        stop=(i == ntk - 1),
                        skip_group_check=True,
                    )
                    del pts[i]

                for i in range(ntk):
                    # interleave filler units evenly across iterations 2..ntk
                    while fi < nf * max(0, i - 1) // span:
                        fillers[fi]()
                        fi += 1
                    diag = i >= 4 * j
                    k = i - 4 * j
                    lo = 128 * k if diag else 0
                    los[i] = lo
                    sp = ps_sc.tile([128, 1024], F32, tag="sc")
                    nc.tensor.matmul(
                        sp[:, lo:512],
                        kT[0:64, ts(i, 128)],
                        qT[0:64, TQW * j + lo:TQW * (j + 1)],
                        start=True,
                        stop=True,
                        skip_group_check=True,
                    )
                    nc.tensor.matmul(
                        sp[:, 512 + lo:1024],
                        kT[64:128, ts(i, 128)],
                        qT[64:128, TQW * j + lo:TQW * (j + 1)],
                        start=True,
                        stop=True,
                        skip_group_check=True,
                    )
                    pt = pt_pool.tile([128, 1024], BF16, tag="pt")
                    if lo == 0:
                        nc.scalar.activation(out=pt[:], in_=sp[:], func=Exp)
                    else:
                        nc.scalar.activation(
                            out=seg2(pt[:], lo, 512 - lo),
                            in_=seg2(sp[:], lo, 512 - lo),
                            func=Exp,
                        )
                    if diag:
                        nc.vector.tensor_mul(
                            out=seg2(pt[:], lo, 128),
                            in0=seg2(pt[:], lo, 128),
                            in1=tm_sb[:],
                        )
                    pts[i] = pt
                    if i >= 2:
                        emit_av(i - 2)
                while fi < nf:
                    fillers[fi]()
                    fi += 1
                emit_av(ntk - 2)
                emit_av(ntk - 1)

                # Tail: softmax denominators -> reciprocal -> broadcast ->
                # norm-muls. Returned as a closure; the main loop emits it
                # as a prefiller of the next window. Mid-run windows use a
                # DMA round-trip broadcast (fully hidden under attention);
                # the final window uses a PE outer-product broadcast so the
                # exposed epilogue chain avoids ~10us of DMA latency.
                def tail(ep=False):
                    # ep=True (final window): the chain is exposed, so its
                    # DMAs ride the now-idle scalar hwdge queue instead of
                    # the sync FIFO that still drains 1MB output writes.
                    dq = nc.scalar if ep else nc.sync
                    r2 = r2_pool.tile([65, 1024], F32, tag="r2")
                    yn = yn_pool.tile([128, TQW], BF16, tag="yn")
                    rd = dram_pool.tile([1, 1024], F32, tag="rd")
                    rb_t = rb_pool.tile([64, 1024], F32, tag="rb")
                    # [8, 128] keeps the gather at 8 DMA descriptors (512B
                    # contiguous runs); [128, 8] would mean 128 tiny writes
                    # and ~4.5us of DMA latency on this chain
                    rsp = r2_pool.tile([8, 128], F32, tag="rsp")
                    r2_tiles[j] = r2
                    nc.vector.tensor_copy(out=r2[64:65, 0:512], in_=yh0[64:65, :])
                    if ep:
                        # scalar engine is idle in the epilogue: run the two
                        # denominator extractions in parallel
                        nc.scalar.copy(out=r2[64:65, 512:1024], in_=yh1[64:65, :])
                    else:
                        nc.vector.tensor_copy(
                            out=r2[64:65, 512:1024], in_=yh1[64:65, :]
                        )
                    dq.dma_start(out=rsp[:], in_=r2[64:65, :])
                    nc.vector.reciprocal(out=rsp[:], in_=rsp[:])
                    dq.dma_start(out=rd[:], in_=rsp[:])
                    bcast_all = bass.AP(
                        tensor=rd.tensor, offset=rd.offset,
                        ap=[[0, 64], [1, 1024]],
                    )
                    dq.dma_start(out=rb_t[:], in_=bcast_all)
                    nc.vector.tensor_mul(
                        out=yn[0:64, :], in0=yh0[0:64, :], in1=rb_t[:, 0:512]
                    )
                    yh = ynhi_pool.tile([64, TQW], BF16, tag="ynhi")
                    nc.vector.tensor_mul(
                        out=yh[:], in0=yh1[0:64, :], in1=rb_t[:, 512:1024]
                    )
                    dq.dma_start(out=yn[64:128, :], in_=yh[:])
                    yn_tiles[j] = yn

                return tail

            # ---- emission ----
            # Window processing order: small window 3 goes LAST so the
            # serial epilogue (final exp drain + tail + proj) is short.
            # Late (ACT-heavy) slots carry the proj fillers; each window's
            # own qT chain runs as a prefiller inside the boundary hole
            # where the PE would otherwise wait for the previous window's
            # exp backlog to free the score psum ping-pong.
            ORDER = [0, 1, 2, 4, 5, 6, 7, 3]
            # slot -> QKV window set up in that slot (rb1/rb2 interleaved;
            # rb0 of the slot's own window is a prefiller)
            QKV_AT = {0: 1, 1: 2, 2: 3, 3: 4, 4: 5, 5: 6, 6: 7}
            # slot -> proj windows run in that slot
            PROJ_AT = {3: [0], 4: [1, 2], 5: [4], 6: [5, 6], 7: [7]}

            xw_tiles[0] = xw_tiles0
            f_xw_dma(1)()
            f_xw_dma(2)()
            f_xw_dma(3)()
            for rb in range(3):
                f_qkv_rb(0, rb, pool=ps_sc, tag="sc")()
            for k in range(4):
                f_vtrans(0, k)()

            tails = {}
            for s, j in enumerate(ORDER):
                pre = []
                fillers = []
                if s >= 1:
                    pre.append(tails[ORDER[s - 1]])
                w = QKV_AT.get(s)
                if w is not None:
                    if w == j:
                        # this window's own qT: must precede its scores
                        pre.append(f_qkv_rb(w, 0))
                    if w + 1 < NW and (w + 1) not in xw_tiles:
                        fillers.append(f_xw_dma(w + 1))
                    if w != j:
                        for q in range(4):
                            fillers.append(f_qkv_rb(w, 0, part=q))
                    for rb in (1, 2):
                        for q in range(4):
                            fillers.append(f_qkv_rb(w, rb, part=q))
                    for k in range(4):
                        fillers.append(f_vtrans(w, k))
                # proj units go in the prefill: they execute inside the
                # boundary hole where the PE otherwise idles on the previous
                # window's exp backlog (and HAM then halves the clock)
                for pw in PROJ_AT.get(s, []):
                    for ot in range(8):
                        pre.append(f_proj(pw, ot))
                tails[j] = emit_attn(j, fillers, prefillers=pre)
            # PE heater part 1: dependency-free dummy matmuls; the
            # scheduler floats these into the final exp-drain region,
            # keeping the clock warm through the last AVs.
            heat = ps_qk.tile([128, TQW], F32, tag="qkv")
            for h in range(20):
                nc.tensor.matmul(
                    heat[:],
                    kT[:, ts(h % 8, 128)],
                    qT[:, 0:TQW],
                    start=True,
                    stop=True,
                    skip_group_check=True,
                )
            tails[ORDER[-1]](ep=True)
            # PE heater part 2: fp32 outer products reading the tail's r2
            # row — the data dependency pins them to run DURING the final
            # tail's DMA/reciprocal chain, which is otherwise the only live
            # work; HAM then keeps K=8/8 and the final projection runs at
            # full clock instead of half.
            r2f = r2_tiles[ORDER[-1]]
            for h in range(12):
                nc.tensor.matmul(
                    heat[0:64, :],
                    ones65[64:65, :],
                    r2f[64:65, 0:512],
                    start=True,
                    stop=True,
                    skip_group_check=True,
                )
            # final window's projection: rotate through three idle psum
            # pools so the psum->SBUF casts never gate the next matmul
            fpools = [(ps_sc, "sc"), (ps_qk, "qkv"), (ps_ev, "ev")]
            for ot in range(8):
                p, t = fpools[ot % 3]
                f_proj(ORDER[-1], ot, pool=p, tag=t)()

    nc.compile()
    return nc


def make_inputs(x, W_qkv, b_qkv, W_proj):
    """Host-side shard prep. Returns in_maps for the 8 cores."""
    s = 1.0 / math.sqrt(64.0)
    xT = np.ascontiguousarray(x.T).astype(ml_dtypes.bfloat16)

    # trimask[r, s, c] = 1 where kept (r <= c), else 0; two identical
    # segments (one per head half)
    r = np.arange(128)[:, None]
    c = np.arange(128)[None, :]
    tm1 = (r <= c).astype(ml_dtypes.bfloat16)  # [128, 128]
    tm = np.concatenate([tm1, tm1], axis=1)  # [128, 256]

    in_maps = []
    for i in range(NCORES):
        sl = slice(128 * i, 128 * i + 128)
        wshard = np.concatenate(
            [W_qkv[0:1024][sl] * s, W_qkv[1024:2048][sl], W_qkv[2048:3072][sl]], axis=0
        )  # [384, 1024]
        wqkvT = np.ascontiguousarray(wshard.T).astype(ml_dtypes.bfloat16)
        bq = np.stack(
            [b_qkv[0:1024][sl] * s, b_qkv[1024:2048][sl], b_qkv[2048:3072][sl]], axis=1
        ).astype(np.float32)  # [128, 3]
        wpT = np.ascontiguousarray(W_proj[:, sl].T).astype(ml_dtypes.bfloat16)
        in_maps.append(
            {"xT": xT, "wqkvT": wqkvT, "bqkv": bq, "wpT": wpT, "trimask": tm}
        )
    return in_maps


_NC_CACHE = {}


def get_nc():
    if "nc" not in _NC_CACHE:
        _NC_CACHE["nc"] = build_nc()
    return _NC_CACHE["nc"]


def kernel(x, W_qkv, b_qkv, W_proj, b_proj):
    x = np.asarray(x, dtype=np.float32)
    W_qkv = np.asarray(W_qkv, dtype=np.float32)
    b_qkv = np.asarray(b_qkv, dtype=np.float32)
    W_proj = np.asarray(W_proj, dtype=np.float32)
    b_proj = np.asarray(b_proj, dtype=np.float32)

    nc = get_nc()
    in_maps = make_inputs(x, W_qkv, b_qkv, W_proj)
    res = run_bass_kernel_spmd(nc, in_maps, core_ids=list(range(NCORES)))
    acc = np.zeros((DM, T), dtype=np.float64)
    for i in range(NCORES):
        acc += res.results[i]["out"].astype(np.float64)
    return (acc.T + b_proj[None, :].astype(np.float64)).astype(np.float32)


if __name__ == "__main__":
    rng = np.random.default_rng(0)
    x = rng.standard_normal((T, DM), dtype=np.float32)
    W_qkv = (rng.standard_normal((3 * DM, DM), dtype=np.float32) / 32.0).astype(
        np.float32
    )
    b_qkv = np.zeros((3 * DM,), dtype=np.float32)
    W_proj = (rng.standard_normal((DM, DM), dtype=np.float32) / 32.0).astype(np.float32)
    b_proj = np.zeros((DM,), dtype=np.float32)
    y = kernel(x, W_qkv, b_qkv, W_proj, b_proj)
    print("kernel output", y.shape, y.dtype)
